# revision 17
# baseline (speedup 1.0000x reference)
"""Trainium2 Bass kernel for nn_DyIntraModalityUpdate (dense transformer block).

Strategy: pure data-parallel over batch (B=32 -> 4 per core x 8 cores); each
core computes both the v- and q- branches for its batches. No collectives.

Per-core program:
  prologue: q masked-means -> q4v gate (scales the v branch); prefetch v weights
  branch v main loop (per batch): x^T via PE transposes (raw f32r + relu
    copies); v-projection token-major (bias via K=1 ones-row matmul); k,q
    projections feature-major ((1+gate)^2 folded into k, token-mask into q);
    per-head-pair: scores^T with K=64 row-split, exp on ACT with -1.25e8
    key-mask bias (softmax without max-subtraction, fp32-safe here), replicated
    row-sums via all-ones lhsT matmul, DVE reciprocal, update^T = v^T @ p with
    partition-shifted DVE eviction, residual add on Pool; out-proj token-major.
    v-means accumulate from the same x tiles; the v4q gate computes at branch
    tail so branch q never waits on a separate mean pass.
  branch q main loop: same, using the v4q gate.

All matmuls run in float32r (TF32-like, 1 cycle/row at N>=256, ~1.5e-4 rel
err). fp32r constraints honored: producers write f32r, moving free >= 2, no
PSUM dst base-partition offsets, one accumulation group per PSUM tile.
"""
import os
import sys

import numpy as np

for _p in ("/opt/trn_rl_repo", "/root/.axon_site/_ro/trn_rl_repo"):
    if os.path.isdir(_p) and _p not in sys.path:
        sys.path.insert(0, _p)

import concourse.bass as bass  # noqa: E402,F401
import concourse.mybir as mybir  # noqa: E402
import concourse.tile as tile  # noqa: E402
from concourse import bacc  # noqa: E402
from concourse.bass_utils import run_bass_kernel_spmd  # noqa: E402
from concourse.masks import make_identity  # noqa: E402

F32 = mybir.dt.float32
F32R = mybir.dt.float32r
ALU = mybir.AluOpType
ACTF = mybir.ActivationFunctionType

B_CORE = 4
NTOK = 256
D = 1024
DQKV = 3 * D
NCORES = 8
NEGBIAS = -1e9 / 8.0  # masked_fill(-1e9) then /sqrt(64)

WEIGHT_NAMES = ("w_v4q", "b_v4q", "w_q4v", "b_q4v",
                "w_vlin", "b_vlin", "w_qlin", "b_qlin",
                "w_vout", "b_vout", "w_qout", "b_qout")


def build_nc():
    nc = bacc.Bacc("TRN2", target_bir_lowering=False, debug=False)
    dram = {}

    def din(name, shape):
        dram[name] = nc.dram_tensor(name, shape, F32, kind="ExternalInput").ap()

    def dout(name, shape):
        dram[name] = nc.dram_tensor(name, shape, F32, kind="ExternalOutput").ap()

    din("v", [B_CORE, NTOK, D])
    din("q", [B_CORE, NTOK, D])
    din("v_mask", [B_CORE, NTOK])
    din("q_mask", [B_CORE, NTOK])
    for g in ("v4q", "q4v"):
        din(f"w_{g}", [D, D])
        din(f"b_{g}", [D])
    for x in ("v", "q"):
        din(f"w_{x}lin", [D, DQKV])
        din(f"b_{x}lin", [DQKV])
        din(f"w_{x}out", [D, D])
        din(f"b_{x}out", [D])
    dout("out_v", [B_CORE, NTOK, D])
    dout("out_q", [B_CORE, NTOK, D])

    with tile.TileContext(nc) as tc:
        with tc.tile_pool(name="cpool", bufs=1) as cpool, \
             tc.tile_pool(name="wpool", bufs=1) as wpool, \
             tc.tile_pool(name="pspool", bufs=8, space="PSUM") as ps:
            # ---- constants ----
            ones_f = cpool.tile([128, 128], F32, name="ones_f")
            nc.gpsimd.memset(ones_f[:], 1.0)
            ones128 = cpool.tile([128, 128], F32R, name="ones128")
            nc.vector.tensor_copy(ones128[:], ones_f[:])
            ones1 = cpool.tile([1, 128], F32R, name="ones1")
            nc.vector.tensor_copy(ones1[:], ones_f[0:1, :])
            ident_f = cpool.tile([128, 128], F32, name="ident_f")
            make_identity(nc, ident_f[:])
            ident = cpool.tile([128, 128], F32R, name="ident")
            nc.vector.tensor_copy(ident[:], ident_f[:])
            zero_f = cpool.tile([128, 4], F32, name="zero_f")
            nc.gpsimd.memset(zero_f[:], 0.0)

            bw = {}

            def load_branch_weights(X):
                wlin_d = dram[f"w_{X}lin"]
                blin_d = dram[f"b_{X}lin"]
                bout_d = dram[f"b_{X}out"]
                wout_d = dram[f"w_{X}out"]
                wl = []
                for kt in range(8):
                    t = wpool.tile([128, DQKV], F32R, name=f"wl_{X}_{kt}",
                                   tag=f"wl{kt}", bufs=1)
                    nc.sync.dma_start(
                        t[:], wlin_d[kt * 128:(kt + 1) * 128, :].bitcast(F32R))
                    wl.append(t)
                b_kq = wpool.tile([128, 16], F32, name=f"bkq_{X}", tag="bkq", bufs=1)
                nc.sync.dma_start(b_kq[:],
                                  blin_d[0:2048].rearrange("(o p) -> p o", p=128))
                b_v = wpool.tile([1, D], F32R, name=f"bv_{X}", tag="bv", bufs=1)
                nc.sync.dma_start(b_v[:], blin_d[2048:3072].bitcast(F32R).unsqueeze(0))
                b_o = wpool.tile([1, D], F32R, name=f"bo_{X}", tag="bo", bufs=1)
                nc.sync.dma_start(b_o[:], bout_d.bitcast(F32R).unsqueeze(0))
                bw[X] = (wl, b_kq, b_v, b_o, wout_d)

            meanT = {}
            g2T = {}

            def emit_gate(pool, gname, dst, src_meanT, tag_prefix):
                """gate = sigmoid(relu(mean) @ w + b); store (1+gate)^2
                transposed as g2T[dst] [128, 8, 4] (fp32, per-partition use)."""
                w_d = dram[f"w_{gname}"]
                b_d = dram[f"b_{gname}"]
                bg = pool.tile([1, D], F32R, name=f"bg_{gname}",
                               tag=f"{tag_prefix}bg", bufs=1)
                nc.sync.dma_start(bg[:], b_d.bitcast(F32R).unsqueeze(0))
                gsb = pool.tile([4, D], F32, name=f"g_{gname}",
                                tag=f"{tag_prefix}gsb", bufs=1)
                psg = [ps.tile([4, 512], F32, name=f"psg_{gname}{h}", tag="ps")
                       for h in range(2)]
                for kt in range(8):
                    for h in range(2):
                        wgt = pool.tile([128, 512], F32R,
                                        name=f"wg_{gname}_{kt}_{h}",
                                        tag=f"{tag_prefix}wg", bufs=2)
                        nc.sync.dma_start(
                            wgt[:], w_d[kt * 128:(kt + 1) * 128,
                                        h * 512:(h + 1) * 512].bitcast(F32R))
                        nc.tensor.matmul(psg[h][:], src_meanT[:, kt, :], wgt[:],
                                         start=(kt == 0), stop=False)
                for h in range(2):
                    nc.tensor.matmul(psg[h][:], ones1[0:1, 0:4],
                                     bg[:, h * 512:(h + 1) * 512],
                                     start=False, stop=True)
                    nc.scalar.activation(gsb[:, h * 512:(h + 1) * 512], psg[h][:],
                                         ACTF.Sigmoid)
                nc.vector.tensor_scalar_add(gsb[:], gsb[:], 1.0)
                g2 = pool.tile([4, D], F32R, name=f"g2_{gname}",
                               tag=("rmv" if tag_prefix == "a" else f"{tag_prefix}g2"), bufs=1)
                nc.vector.tensor_tensor(g2[:], gsb[:], gsb[:], ALU.mult)
                gt = wpool.tile([128, 8, 4], F32, name=f"g2T_{dst}")
                for c in range(8):
                    pst = ps.tile([128, 4], F32R, name=f"psgt_{gname}{c}", tag="ps")
                    nc.tensor.transpose(pst[:], g2[:, c * 128:(c + 1) * 128],
                                        ident[0:4, 0:4])
                    nc.vector.tensor_copy(gt[:, c, :], pst[:])
                g2T[dst] = gt

            # ---- prologue: q masked-means -> q4v gate (needed by branch v) ----
            with tc.tile_pool(name="propool", bufs=1) as pp:
                m_d = dram["q_mask"]
                x_d = dram["q"]
                ps_mean = [ps.tile([4, 512], F32, name=f"psmean_q{h}", tag="ps")
                           for h in range(2)]
                ps_n = ps.tile([4, 2], F32, name="psn_q", tag="ps")
                for b in range(B_CORE):
                    for jt in range(2):
                        xt = pp.tile([128, D], F32R, name=f"mx_q_{b}_{jt}",
                                     tag="mx", bufs=2)
                        nc.sync.dma_start(
                            xt[:], x_d[b, jt * 128:(jt + 1) * 128, :].bitcast(F32R))
                        mc = pp.tile([128, 4], F32R, name=f"mc_q_{b}_{jt}",
                                     tag="mc", bufs=4)
                        nc.vector.tensor_copy(mc[:], zero_f[:])
                        nc.sync.dma_start(
                            mc[:, b:b + 1],
                            m_d[b, jt * 128:(jt + 1) * 128].bitcast(F32R).unsqueeze(1))
                        first = (b == 0 and jt == 0)
                        last = (b == B_CORE - 1 and jt == 1)
                        for h in range(2):
                            nc.tensor.matmul(ps_mean[h][:], mc[:],
                                             xt[:, h * 512:(h + 1) * 512],
                                             start=first, stop=last)
                        nc.tensor.matmul(ps_n[:], mc[:], ones128[:, 0:2],
                                         start=first, stop=last)
                recn = pp.tile([4, 1], F32, name="recn_q", tag="recn", bufs=1)
                nc.vector.reciprocal(recn[:], ps_n[:, 0:1])
                rmean = pp.tile([4, D], F32R, name="rmean_q", tag="rmean", bufs=1)
                for h in range(2):
                    # relu(masked_sum / n): (psum * recn) max 0
                    nc.vector.tensor_scalar(rmean[:, h * 512:(h + 1) * 512],
                                            ps_mean[h][:], recn[:], 0.0,
                                            ALU.mult, ALU.max)
                mt = wpool.tile([128, 8, 4], F32R, name="meanT_q")
                for c in range(8):
                    pst = ps.tile([128, 4], F32R, name=f"psmt_q{c}", tag="ps")
                    nc.tensor.transpose(pst[:], rmean[:, c * 128:(c + 1) * 128],
                                        ident[0:4, 0:4])
                    nc.vector.tensor_copy(mt[:, c, :], pst[:])
                meanT["q"] = mt

                # q4v gate scales branch v
                emit_gate(pp, "q4v", "v", meanT["q"], "p")
                # prefetch branch-v main weights during gate compute
                load_branch_weights("v")

            # ---- main: per branch ----
            apool_ctx = tc.tile_pool(name="apool", bufs=1)
            apool = apool_ctx.__enter__()
            for X in ("v", "q"):
                gate = g2T[X]
                x_d = dram[X]
                m_d = dram[f"{X}_mask"]
                out_d = dram[f"out_{X}"]
                if X not in bw:
                    load_branch_weights(X)
                wl, b_kq, b_v, b_o, wout_d = bw[X]

                fold_means = (X == "v")
                if fold_means:
                    accv = apool.tile([4, D], F32, name="accv", tag="accv", bufs=1)
                    accn = apool.tile([4, 2], F32, name="accn", tag="accn", bufs=1)

                for b in range(B_CORE):
                    # loads
                    xt = []
                    for jt in range(2):
                        t = apool.tile([128, D], F32R, name=f"x_{X}_{b}_{jt}",
                                       tag="xt", bufs=2)
                        nc.sync.dma_start(
                            t[:], x_d[b, jt * 128:(jt + 1) * 128, :].bitcast(F32R))
                        xt.append(t)
                    mrow = apool.tile([1, NTOK], F32R, name=f"mrow_{X}_{b}",
                                      tag="mrow", bufs=1)
                    nc.sync.dma_start(mrow[:], m_d[b].bitcast(F32R).unsqueeze(0))
                    psmr = ps.tile([128, NTOK], F32, name=f"psmr_{X}_{b}", tag="ps")
                    nc.tensor.matmul(psmr[:], ones1[:], mrow[:], start=True, stop=True)
                    maskrep = apool.tile([128, NTOK], F32, name=f"maskrep_{X}_{b}",
                                         tag="maskrep", bufs=1)
                    nc.vector.tensor_copy(maskrep[:], psmr[:])
                    mb = []
                    for jt in range(2):
                        mcol = apool.tile([128, 1], F32, name=f"mcol_{X}_{b}_{jt}",
                                          tag="mcol", bufs=4)
                        nc.sync.dma_start(
                            mcol[:], m_d[b, jt * 128:(jt + 1) * 128].unsqueeze(1))
                        t = apool.tile([128, 1], F32, name=f"mbias_{X}_{b}_{jt}",
                                       tag="mbias", bufs=4)
                        nc.vector.tensor_scalar(t[:], mcol[:], 1.0, -NEGBIAS,
                                                ALU.subtract, ALU.mult)
                        mb.append(t)

                    if fold_means:
                        # accumulate v masked-sums from this batch's x tiles
                        pm = [ps.tile([4, 512], F32, name=f"pmv_{b}{h}", tag="ps")
                              for h in range(2)]
                        pn = ps.tile([4, 2], F32, name=f"pnv_{b}", tag="ps")
                        for jt in range(2):
                            mc = apool.tile([128, 4], F32R, name=f"mcv_{b}_{jt}",
                                            tag="mcv", bufs=4)
                            nc.vector.tensor_copy(mc[:], zero_f[:])
                            nc.sync.dma_start(
                                mc[:, b:b + 1],
                                m_d[b, jt * 128:(jt + 1) * 128]
                                .bitcast(F32R).unsqueeze(1))
                            for h in range(2):
                                nc.tensor.matmul(pm[h][:], mc[:],
                                                 xt[jt][:, h * 512:(h + 1) * 512],
                                                 start=(jt == 0), stop=(jt == 1))
                            nc.tensor.matmul(pn[:], mc[:], ones128[:, 0:2],
                                             start=(jt == 0), stop=(jt == 1))
                        if b == 0:
                            for h in range(2):
                                nc.vector.tensor_copy(
                                    accv[:, h * 512:(h + 1) * 512], pm[h][:])
                            nc.vector.tensor_copy(accn[:], pn[:])
                        else:
                            for h in range(2):
                                nc.vector.tensor_tensor(
                                    accv[:, h * 512:(h + 1) * 512],
                                    accv[:, h * 512:(h + 1) * 512], pm[h][:],
                                    ALU.add)
                            nc.vector.tensor_tensor(accn[:], accn[:], pn[:], ALU.add)

                    # transpose x -> feature-major raw + relu copies
                    xTraw = apool.tile([128, 8, NTOK], F32R, name=f"xTraw_{X}_{b}",
                                       tag="xTraw", bufs=1)
                    xTrelu = apool.tile([128, 8, NTOK], F32R, name=f"xTrelu_{X}_{b}",
                                        tag="xTrelu", bufs=1)
                    for jt in range(2):
                        for c in range(8):
                            pst = ps.tile([128, 128], F32R,
                                          name=f"pstp_{X}_{b}_{jt}_{c}", tag="ps")
                            nc.tensor.transpose(pst[:],
                                                xt[jt][:, c * 128:(c + 1) * 128],
                                                ident[:])
                            nc.vector.tensor_copy(
                                xTraw[:, c, jt * 128:(jt + 1) * 128], pst[:])
                            nc.scalar.activation(
                                xTrelu[:, c, jt * 128:(jt + 1) * 128], pst[:],
                                ACTF.Relu)

                    # v projection: token-major [tok, dout], bias via ones-row mm
                    vtok = []
                    for jt in range(2):
                        vt = apool.tile([128, D], F32R, name=f"vtok_{X}_{b}_{jt}",
                                        tag=f"vtok{jt}", bufs=1)
                        vtok.append(vt)
                    for jt in range(2):
                        for ch in range(2):
                            psv = ps.tile([128, 512], F32,
                                          name=f"psv_{X}_{b}_{jt}_{ch}", tag="ps")
                            for kt in range(8):
                                nc.tensor.matmul(
                                    psv[:], xTrelu[:, kt, jt * 128:(jt + 1) * 128],
                                    wl[kt][:, 2048 + ch * 512:2048 + (ch + 1) * 512],
                                    start=(kt == 0), stop=False)
                            nc.tensor.matmul(psv[:], ones1[:],
                                             b_v[:, ch * 512:(ch + 1) * 512],
                                             start=False, stop=True)
                            nc.scalar.copy(vtok[jt][:, ch * 512:(ch + 1) * 512],
                                           psv[:])

                    # k,q projections for all head pairs (dense PE phase)
                    k_ts, q_ts = {}, {}
                    for mp in range(8):
                        for part in (mp, 8 + mp):  # k chunk then q chunk
                            psq = ps.tile([128, NTOK], F32,
                                          name=f"pskq_{X}_{b}_{part}", tag="ps")
                            for kt in range(8):
                                nc.tensor.matmul(
                                    psq[:], wl[kt][:, part * 128:(part + 1) * 128],
                                    xTrelu[:, kt, :], start=(kt == 0), stop=(kt == 7))
                            if part < 8:
                                t = apool.tile([128, NTOK], F32R,
                                               name=f"k_{X}_{b}_{mp}", tag=f"k{mp}",
                                               bufs=1)
                                # (psum + bias) * (1+gate)^2  [both per-partition]
                                nc.vector.tensor_scalar(
                                    t[:], psq[:], b_kq[:, part:part + 1],
                                    gate[:, part, b:b + 1], ALU.add, ALU.mult)
                                k_ts[mp] = t
                            else:
                                t = apool.tile([128, NTOK], F32R,
                                               name=f"q_{X}_{b}_{mp}", tag=f"q{mp}",
                                               bufs=1)
                                # (psum + bias) * token_mask  [mask replicated]
                                nc.vector.scalar_tensor_tensor(
                                    t[:], psq[:], b_kq[:, part:part + 1], maskrep[:],
                                    ALU.add, ALU.mult)
                                q_ts[mp] = t

                    # attention per head pair
                    for mp in range(8):
                        k_t, q_t = k_ts[mp], q_ts[mp]
                        pT_mp = []
                        for jt in range(2):
                            pt2 = apool.tile([128, 512], F32R,
                                             name=f"pT_{X}_{b}_{mp}_{jt}", tag="pT",
                                             bufs=4)
                            for h_loc in range(2):
                                r0 = h_loc * 64
                                pss = ps.tile([128, NTOK], F32,
                                              name=f"pss_{X}_{b}_{mp}_{jt}_{h_loc}",
                                              tag="ps")
                                nc.tensor.matmul(
                                    pss[:], k_t[r0:r0 + 64, jt * 128:(jt + 1) * 128],
                                    q_t[r0:r0 + 64, :], start=True, stop=True)
                                nc.scalar.activation(
                                    pt2[:, h_loc * 256:(h_loc + 1) * 256], pss[:],
                                    ACTF.Exp, bias=mb[jt][:], scale=0.125)
                            pT_mp.append(pt2)

                        # replicated row-sums + reciprocal
                        psr = ps.tile([128, 512], F32, name=f"psr_{X}_{b}_{mp}",
                                      tag="ps")
                        nc.tensor.matmul(psr[:], ones128[:], pT_mp[0][:],
                                         start=True, stop=False)
                        nc.tensor.matmul(psr[:], ones128[:], pT_mp[1][:],
                                         start=False, stop=True)
                        rinv = apool.tile([128, 512], F32, name=f"rinv_{X}_{b}_{mp}",
                                          tag="rinv", bufs=2)
                        nc.vector.reciprocal(rinv[:], psr[:])

                        # update^T = v^T @ p (one [64,256] psum per head;
                        # partition-shifted DVE eviction into the pair tile)
                        u_tmp = apool.tile([128, NTOK], F32, name=f"ut_{X}_{b}_{mp}",
                                           tag="utmp", bufs=2)
                        for h_loc in range(2):
                            h = 2 * mp + h_loc
                            psu = ps.tile([64, NTOK], F32,
                                          name=f"psu_{X}_{b}_{mp}_{h_loc}", tag="ps")
                            for jt in range(2):
                                nc.tensor.matmul(
                                    psu[:],
                                    vtok[jt][:, h * 64:(h + 1) * 64],
                                    pT_mp[jt][:, h_loc * 256:(h_loc + 1) * 256],
                                    start=(jt == 0), stop=(jt == 1))
                            r0 = h_loc * 64
                            nc.vector.tensor_tensor(
                                u_tmp[r0:r0 + 64, :], psu[0:64, :],
                                rinv[0:64, h_loc * 256:(h_loc + 1) * 256],
                                ALU.mult)
                        # residual: x^T += u^T (in place, on the idle Pool engine)
                        nc.gpsimd.tensor_tensor(xTraw[:, mp, :], xTraw[:, mp, :],
                                                u_tmp[:], ALU.add)

                    # output projection (w_out resident) -> ACT evict -> DMA out
                    pso = [ps.tile([128, 512], F32, name=f"pso_{X}_{b}_{i}", tag="ps")
                           for i in range(4)]
                    for kt in range(8):
                        wo = apool.tile([128, D], F32R, name=f"wo_{X}_{b}_{kt}",
                                        tag="wo", bufs=3)
                        nc.sync.dma_start(
                            wo[:], wout_d[kt * 128:(kt + 1) * 128, :].bitcast(F32R))
                        for i in range(4):
                            it, ch = divmod(i, 2)
                            nc.tensor.matmul(pso[i][:],
                                             xTraw[:, kt, it * 128:(it + 1) * 128],
                                             wo[:, ch * 512:(ch + 1) * 512],
                                             start=(kt == 0), stop=False)
                    for i in range(4):
                        it, ch = divmod(i, 2)
                        nc.tensor.matmul(pso[i][:], ones1[:],
                                         b_o[:, ch * 512:(ch + 1) * 512],
                                         start=False, stop=True)
                        osb = apool.tile([128, 512], F32, name=f"osb_{X}_{b}_{i}",
                                         tag="osb", bufs=2)
                        nc.scalar.copy(osb[:], pso[i][:])
                        nc.sync.dma_start(
                            out_d[b, it * 128:(it + 1) * 128, ch * 512:(ch + 1) * 512],
                            osb[:])

                if fold_means:
                    # finish v means and compute the v4q gate for branch q
                    recn = apool.tile([4, 1], F32, name="recn_v", tag="recnv", bufs=1)
                    nc.vector.reciprocal(recn[:], accn[:, 0:1])
                    rmean = apool.tile([4, D], F32R, name="rmean_v", tag="rmv",
                                       bufs=1)
                    nc.vector.tensor_scalar(rmean[:], accv[:], recn[:], 0.0,
                                            ALU.mult, ALU.max)
                    mt = wpool.tile([128, 8, 4], F32R, name="meanT_v")
                    for c in range(8):
                        pst = ps.tile([128, 4], F32R, name=f"psmt_v{c}", tag="ps")
                        nc.tensor.transpose(pst[:], rmean[:, c * 128:(c + 1) * 128],
                                            ident[0:4, 0:4])
                        nc.vector.tensor_copy(mt[:, c, :], pst[:])
                    meanT["v"] = mt
                    emit_gate(apool, "v4q", "q", meanT["v"], "a")
            apool_ctx.__exit__(None, None, None)
    nc.compile()
    return nc


_NC = None


def _get_nc():
    global _NC
    if _NC is None:
        _NC = build_nc()
    return _NC


def run(inputs, trace=False):
    nc = _get_nc()
    in_maps = []
    for c in range(NCORES):
        sl = slice(B_CORE * c, B_CORE * (c + 1))
        m = {"v": np.ascontiguousarray(np.asarray(inputs["v"], dtype=np.float32)[sl]),
             "q": np.ascontiguousarray(np.asarray(inputs["q"], dtype=np.float32)[sl]),
             "v_mask": np.ascontiguousarray(
                 np.asarray(inputs["v_mask"], dtype=np.float32)[sl]),
             "q_mask": np.ascontiguousarray(
                 np.asarray(inputs["q_mask"], dtype=np.float32)[sl])}
        for name in WEIGHT_NAMES:
            m[name] = np.ascontiguousarray(np.asarray(inputs[name], dtype=np.float32))
        in_maps.append(m)
    res = run_bass_kernel_spmd(nc, in_maps, core_ids=list(range(NCORES)),
                               trace=trace)
    uv = np.concatenate([res.results[c]["out_v"] for c in range(NCORES)], axis=0)
    uq = np.concatenate([res.results[c]["out_q"] for c in range(NCORES)], axis=0)
    return (uv, uq), res


def kernel(**inputs):
    (uv, uq), _ = run(inputs, trace=False)
    return uv, uq


# revision 21
# speedup vs baseline: 146.5628x; 146.5628x over previous
"""Trainium2 Bass kernel for nn_DyIntraModalityUpdate (dense transformer block).

Strategy: pure data-parallel over batch (B=32 -> 4 per core x 8 cores); each
core computes both the v- and q- branches for its batches. No collectives.

Per-core program:
  prologue: q masked-means -> q4v gate (scales the v branch); prefetch v weights
  branch v main loop (per batch): x^T via PE transposes (raw f32r + relu
    copies); v-projection token-major (bias via K=1 ones-row matmul); k,q
    projections feature-major ((1+gate)^2 folded into k, token-mask into q);
    per-head-pair: scores^T with K=64 row-split, exp on ACT with -1.25e8
    key-mask bias (softmax without max-subtraction, fp32-safe here), replicated
    row-sums via all-ones lhsT matmul, DVE reciprocal, update^T = v^T @ p with
    partition-shifted DVE eviction, residual add on Pool; out-proj token-major.
    v-means accumulate from the same x tiles; the v4q gate computes at branch
    tail so branch q never waits on a separate mean pass.
  branch q main loop: same, using the v4q gate.

All matmuls run in float32r (TF32-like, 1 cycle/row at N>=256, ~1.5e-4 rel
err). fp32r constraints honored: producers write f32r, moving free >= 2, no
PSUM dst base-partition offsets, one accumulation group per PSUM tile.
"""
import os
import sys

import numpy as np

for _p in ("/opt/trn_rl_repo", "/root/.axon_site/_ro/trn_rl_repo"):
    if os.path.isdir(_p) and _p not in sys.path:
        sys.path.insert(0, _p)

import concourse.bass as bass  # noqa: E402,F401
import concourse.mybir as mybir  # noqa: E402
import concourse.tile as tile  # noqa: E402
from concourse import bacc  # noqa: E402
from concourse.bass_utils import run_bass_kernel_spmd  # noqa: E402
from concourse.masks import make_identity  # noqa: E402

F32 = mybir.dt.float32
F32R = mybir.dt.float32r
ALU = mybir.AluOpType
ACTF = mybir.ActivationFunctionType

B_CORE = 4
NTOK = 256
D = 1024
DQKV = 3 * D
NCORES = 8
NEGBIAS = -1e9 / 8.0  # masked_fill(-1e9) then /sqrt(64)

WEIGHT_NAMES = ("w_v4q", "b_v4q", "w_q4v", "b_q4v",
                "w_vlin", "b_vlin", "w_qlin", "b_qlin",
                "w_vout", "b_vout", "w_qout", "b_qout")


def build_nc():
    nc = bacc.Bacc("TRN2", target_bir_lowering=False, debug=False)
    dram = {}

    def din(name, shape):
        dram[name] = nc.dram_tensor(name, shape, F32, kind="ExternalInput").ap()

    def dout(name, shape):
        dram[name] = nc.dram_tensor(name, shape, F32, kind="ExternalOutput").ap()

    din("v", [B_CORE, NTOK, D])
    din("q", [B_CORE, NTOK, D])
    din("v_mask", [B_CORE, NTOK])
    din("q_mask", [B_CORE, NTOK])
    for g in ("v4q", "q4v"):
        din(f"w_{g}", [D, D])
        din(f"b_{g}", [D])
    for x in ("v", "q"):
        din(f"w_{x}lin", [D, DQKV])
        din(f"b_{x}lin", [DQKV])
        din(f"w_{x}out", [D, D])
        din(f"b_{x}out", [D])
    dout("out_v", [B_CORE, NTOK, D])
    dout("out_q", [B_CORE, NTOK, D])

    with tile.TileContext(nc) as tc:
        with tc.tile_pool(name="cpool", bufs=1) as cpool, \
             tc.tile_pool(name="wpool", bufs=1) as wpool, \
             tc.tile_pool(name="pspool", bufs=8, space="PSUM") as ps:
            # ---- constants ----
            ones_f = cpool.tile([128, 128], F32, name="ones_f")
            nc.gpsimd.memset(ones_f[:], 1.0)
            ones128 = cpool.tile([128, 128], F32R, name="ones128")
            nc.vector.tensor_copy(ones128[:], ones_f[:])
            ones1 = cpool.tile([1, 128], F32R, name="ones1")
            nc.vector.tensor_copy(ones1[:], ones_f[0:1, :])
            ident_f = cpool.tile([128, 128], F32, name="ident_f")
            make_identity(nc, ident_f[:])
            ident = cpool.tile([128, 128], F32R, name="ident")
            nc.vector.tensor_copy(ident[:], ident_f[:])
            zero_f = cpool.tile([128, 4], F32, name="zero_f")
            nc.gpsimd.memset(zero_f[:], 0.0)

            bw = {}

            def load_branch_weights(X):
                wlin_d = dram[f"w_{X}lin"]
                blin_d = dram[f"b_{X}lin"]
                bout_d = dram[f"b_{X}out"]
                wout_d = dram[f"w_{X}out"]
                wl = []
                for kt in range(8):
                    t = wpool.tile([128, DQKV], F32R, name=f"wl_{X}_{kt}",
                                   tag=f"wl{kt}", bufs=1)
                    nc.sync.dma_start(
                        t[:], wlin_d[kt * 128:(kt + 1) * 128, :].bitcast(F32R))
                    wl.append(t)
                b_kq = wpool.tile([128, 16], F32, name=f"bkq_{X}", tag="bkq", bufs=1)
                nc.sync.dma_start(b_kq[:],
                                  blin_d[0:2048].rearrange("(o p) -> p o", p=128))
                b_v = wpool.tile([1, D], F32R, name=f"bv_{X}", tag="bv", bufs=1)
                nc.sync.dma_start(b_v[:], blin_d[2048:3072].bitcast(F32R).unsqueeze(0))
                b_o = wpool.tile([1, D], F32R, name=f"bo_{X}", tag="bo", bufs=1)
                nc.sync.dma_start(b_o[:], bout_d.bitcast(F32R).unsqueeze(0))
                bw[X] = (wl, b_kq, b_v, b_o, wout_d)

            meanT = {}
            g2T = {}

            def emit_gate(pool, gname, dst, src_meanT, tag_prefix):
                """gate = sigmoid(relu(mean) @ w + b); store (1+gate)^2
                transposed as g2T[dst] [128, 8, 4] (fp32, per-partition use)."""
                w_d = dram[f"w_{gname}"]
                b_d = dram[f"b_{gname}"]
                bg = pool.tile([1, D], F32R, name=f"bg_{gname}",
                               tag=f"{tag_prefix}bg", bufs=1)
                nc.sync.dma_start(bg[:], b_d.bitcast(F32R).unsqueeze(0))
                gsb = pool.tile([4, D], F32, name=f"g_{gname}",
                                tag=f"{tag_prefix}gsb", bufs=1)
                psg = [ps.tile([4, 512], F32, name=f"psg_{gname}{h}", tag="ps")
                       for h in range(2)]
                for kt in range(8):
                    for h in range(2):
                        wgt = pool.tile([128, 512], F32R,
                                        name=f"wg_{gname}_{kt}_{h}",
                                        tag=f"{tag_prefix}wg", bufs=2)
                        nc.sync.dma_start(
                            wgt[:], w_d[kt * 128:(kt + 1) * 128,
                                        h * 512:(h + 1) * 512].bitcast(F32R))
                        nc.tensor.matmul(psg[h][:], src_meanT[:, kt, :], wgt[:],
                                         start=(kt == 0), stop=False)
                for h in range(2):
                    nc.tensor.matmul(psg[h][:], ones1[0:1, 0:4],
                                     bg[:, h * 512:(h + 1) * 512],
                                     start=False, stop=True)
                    nc.scalar.activation(gsb[:, h * 512:(h + 1) * 512], psg[h][:],
                                         ACTF.Sigmoid)
                nc.vector.tensor_scalar_add(gsb[:], gsb[:], 1.0)
                g2 = pool.tile([4, D], F32R, name=f"g2_{gname}",
                               tag=("rmv" if tag_prefix == "a" else f"{tag_prefix}g2"), bufs=1)
                nc.vector.tensor_tensor(g2[:], gsb[:], gsb[:], ALU.mult)
                gt = wpool.tile([128, 8, 4], F32, name=f"g2T_{dst}")
                for c in range(8):
                    pst = ps.tile([128, 4], F32R, name=f"psgt_{gname}{c}", tag="ps")
                    nc.tensor.transpose(pst[:], g2[:, c * 128:(c + 1) * 128],
                                        ident[0:4, 0:4])
                    nc.vector.tensor_copy(gt[:, c, :], pst[:])
                g2T[dst] = gt

            # ---- prologue: q masked-means -> q4v gate (needed by branch v) ----
            with tc.tile_pool(name="propool", bufs=1) as pp:
                m_d = dram["q_mask"]
                x_d = dram["q"]
                ps_mean = [ps.tile([4, 512], F32, name=f"psmean_q{h}", tag="ps")
                           for h in range(2)]
                ps_n = ps.tile([4, 2], F32, name="psn_q", tag="ps")
                for b in range(B_CORE):
                    for jt in range(2):
                        xt = pp.tile([128, D], F32R, name=f"mx_q_{b}_{jt}",
                                     tag="mx", bufs=2)
                        nc.sync.dma_start(
                            xt[:], x_d[b, jt * 128:(jt + 1) * 128, :].bitcast(F32R))
                        mc = pp.tile([128, 4], F32R, name=f"mc_q_{b}_{jt}",
                                     tag="mc", bufs=4)
                        nc.vector.tensor_copy(mc[:], zero_f[:])
                        nc.sync.dma_start(
                            mc[:, b:b + 1],
                            m_d[b, jt * 128:(jt + 1) * 128].bitcast(F32R).unsqueeze(1))
                        first = (b == 0 and jt == 0)
                        last = (b == B_CORE - 1 and jt == 1)
                        for h in range(2):
                            nc.tensor.matmul(ps_mean[h][:], mc[:],
                                             xt[:, h * 512:(h + 1) * 512],
                                             start=first, stop=last)
                        nc.tensor.matmul(ps_n[:], mc[:], ones128[:, 0:2],
                                         start=first, stop=last)
                recn = pp.tile([4, 1], F32, name="recn_q", tag="recn", bufs=1)
                nc.vector.reciprocal(recn[:], ps_n[:, 0:1])
                rmean = pp.tile([4, D], F32R, name="rmean_q", tag="rmean", bufs=1)
                for h in range(2):
                    # relu(masked_sum / n): (psum * recn) max 0
                    nc.vector.tensor_scalar(rmean[:, h * 512:(h + 1) * 512],
                                            ps_mean[h][:], recn[:], 0.0,
                                            ALU.mult, ALU.max)
                mt = wpool.tile([128, 8, 4], F32R, name="meanT_q")
                for c in range(8):
                    pst = ps.tile([128, 4], F32R, name=f"psmt_q{c}", tag="ps")
                    nc.tensor.transpose(pst[:], rmean[:, c * 128:(c + 1) * 128],
                                        ident[0:4, 0:4])
                    nc.vector.tensor_copy(mt[:, c, :], pst[:])
                meanT["q"] = mt

                # q4v gate scales branch v
                emit_gate(pp, "q4v", "v", meanT["q"], "p")
                # prefetch branch-v main weights during gate compute
                load_branch_weights("v")

            # ---- main: per branch ----
            apool_ctx = tc.tile_pool(name="apool", bufs=1)
            apool = apool_ctx.__enter__()
            for X in ("v", "q"):
                gate = g2T[X]
                x_d = dram[X]
                m_d = dram[f"{X}_mask"]
                out_d = dram[f"out_{X}"]
                if X not in bw:
                    load_branch_weights(X)
                wl, b_kq, b_v, b_o, wout_d = bw[X]

                fold_means = (X == "v")
                if fold_means:
                    accv = apool.tile([4, D], F32, name="accv", tag="accv", bufs=1)
                    accn = apool.tile([4, 2], F32, name="accn", tag="accn", bufs=1)

                for b in range(B_CORE):
                    # loads
                    xt = []
                    for jt in range(2):
                        t = apool.tile([128, D], F32R, name=f"x_{X}_{b}_{jt}",
                                       tag="xt", bufs=2)
                        nc.sync.dma_start(
                            t[:], x_d[b, jt * 128:(jt + 1) * 128, :].bitcast(F32R))
                        xt.append(t)
                    mrow = apool.tile([1, NTOK], F32R, name=f"mrow_{X}_{b}",
                                      tag="mrow", bufs=1)
                    nc.sync.dma_start(mrow[:], m_d[b].bitcast(F32R).unsqueeze(0))
                    psmr = ps.tile([128, NTOK], F32, name=f"psmr_{X}_{b}", tag="ps")
                    nc.tensor.matmul(psmr[:], ones1[:], mrow[:], start=True, stop=True)
                    maskrep = apool.tile([128, NTOK], F32, name=f"maskrep_{X}_{b}",
                                         tag="maskrep", bufs=1)
                    nc.vector.tensor_copy(maskrep[:], psmr[:])
                    mb = []
                    for jt in range(2):
                        mcol = apool.tile([128, 1], F32, name=f"mcol_{X}_{b}_{jt}",
                                          tag="mcol", bufs=4)
                        nc.sync.dma_start(
                            mcol[:], m_d[b, jt * 128:(jt + 1) * 128].unsqueeze(1))
                        t = apool.tile([128, 1], F32, name=f"mbias_{X}_{b}_{jt}",
                                       tag="mbias", bufs=4)
                        nc.vector.tensor_scalar(t[:], mcol[:], 1.0, -NEGBIAS,
                                                ALU.subtract, ALU.mult)
                        mb.append(t)

                    if fold_means:
                        # accumulate v masked-sums from this batch's x tiles
                        pm = [ps.tile([4, 512], F32, name=f"pmv_{b}{h}", tag="ps")
                              for h in range(2)]
                        pn = ps.tile([4, 2], F32, name=f"pnv_{b}", tag="ps")
                        for jt in range(2):
                            mc = apool.tile([128, 4], F32R, name=f"mcv_{b}_{jt}",
                                            tag="mcv", bufs=4)
                            nc.vector.tensor_copy(mc[:], zero_f[:])
                            nc.sync.dma_start(
                                mc[:, b:b + 1],
                                m_d[b, jt * 128:(jt + 1) * 128]
                                .bitcast(F32R).unsqueeze(1))
                            for h in range(2):
                                nc.tensor.matmul(pm[h][:], mc[:],
                                                 xt[jt][:, h * 512:(h + 1) * 512],
                                                 start=(jt == 0), stop=(jt == 1))
                            nc.tensor.matmul(pn[:], mc[:], ones128[:, 0:2],
                                             start=(jt == 0), stop=(jt == 1))
                        if b == 0:
                            for h in range(2):
                                nc.vector.tensor_copy(
                                    accv[:, h * 512:(h + 1) * 512], pm[h][:])
                            nc.vector.tensor_copy(accn[:], pn[:])
                        else:
                            for h in range(2):
                                nc.vector.tensor_tensor(
                                    accv[:, h * 512:(h + 1) * 512],
                                    accv[:, h * 512:(h + 1) * 512], pm[h][:],
                                    ALU.add)
                            nc.vector.tensor_tensor(accn[:], accn[:], pn[:], ALU.add)

                    # transpose x -> feature-major raw + relu copies
                    xTraw = apool.tile([128, 8, NTOK], F32R, name=f"xTraw_{X}_{b}",
                                       tag="xTraw", bufs=1)
                    xTrelu = apool.tile([128, 8, NTOK], F32R, name=f"xTrelu_{X}_{b}",
                                        tag="xTrelu", bufs=1)
                    for jt in range(2):
                        for c in range(8):
                            pst = ps.tile([128, 128], F32R,
                                          name=f"pstp_{X}_{b}_{jt}_{c}", tag="ps")
                            nc.tensor.transpose(pst[:],
                                                xt[jt][:, c * 128:(c + 1) * 128],
                                                ident[:])
                            nc.vector.tensor_copy(
                                xTraw[:, c, jt * 128:(jt + 1) * 128], pst[:])
                            nc.scalar.activation(
                                xTrelu[:, c, jt * 128:(jt + 1) * 128], pst[:],
                                ACTF.Relu)

                    # v projection: token-major [tok, dout], bias via ones-row mm
                    vtok = []
                    for jt in range(2):
                        vt = apool.tile([128, D], F32R, name=f"vtok_{X}_{b}_{jt}",
                                        tag=f"vtok{jt}", bufs=1)
                        vtok.append(vt)
                    for jt in range(2):
                        for ch in range(2):
                            psv = ps.tile([128, 512], F32,
                                          name=f"psv_{X}_{b}_{jt}_{ch}", tag="ps")
                            for kt in range(8):
                                nc.tensor.matmul(
                                    psv[:], xTrelu[:, kt, jt * 128:(jt + 1) * 128],
                                    wl[kt][:, 2048 + ch * 512:2048 + (ch + 1) * 512],
                                    start=(kt == 0), stop=False)
                            nc.tensor.matmul(psv[:], ones1[:],
                                             b_v[:, ch * 512:(ch + 1) * 512],
                                             start=False, stop=True)
                            nc.scalar.copy(vtok[jt][:, ch * 512:(ch + 1) * 512],
                                           psv[:])

                    # k,q projections for all head pairs (dense PE phase)
                    k_ts, q_ts = {}, {}
                    for mp in range(8):
                        for part in (mp, 8 + mp):  # k chunk then q chunk
                            psq = ps.tile([128, NTOK], F32,
                                          name=f"pskq_{X}_{b}_{part}", tag="ps")
                            for kt in range(8):
                                nc.tensor.matmul(
                                    psq[:], wl[kt][:, part * 128:(part + 1) * 128],
                                    xTrelu[:, kt, :], start=(kt == 0), stop=(kt == 7))
                            if part < 8:
                                t = apool.tile([128, NTOK], F32R,
                                               name=f"k_{X}_{b}_{mp}", tag=f"k{mp}",
                                               bufs=1)
                                # (psum + bias) * (1+gate)^2  [both per-partition]
                                nc.vector.tensor_scalar(
                                    t[:], psq[:], b_kq[:, part:part + 1],
                                    gate[:, part, b:b + 1], ALU.add, ALU.mult)
                                k_ts[mp] = t
                            else:
                                t = apool.tile([128, NTOK], F32R,
                                               name=f"q_{X}_{b}_{mp}", tag=f"q{mp}",
                                               bufs=1)
                                # (psum + bias) * token_mask  [mask replicated]
                                nc.vector.scalar_tensor_tensor(
                                    t[:], psq[:], b_kq[:, part:part + 1], maskrep[:],
                                    ALU.add, ALU.mult)
                                q_ts[mp] = t

                    # attention per head pair
                    for mp in range(8):
                        k_t, q_t = k_ts[mp], q_ts[mp]
                        pT_mp = []
                        for jt in range(2):
                            pt2 = apool.tile([128, 512], F32R,
                                             name=f"pT_{X}_{b}_{mp}_{jt}", tag="pT",
                                             bufs=4)
                            for h_loc in range(2):
                                r0 = h_loc * 64
                                pss = ps.tile([128, NTOK], F32,
                                              name=f"pss_{X}_{b}_{mp}_{jt}_{h_loc}",
                                              tag="ps")
                                nc.tensor.matmul(
                                    pss[:], k_t[r0:r0 + 64, jt * 128:(jt + 1) * 128],
                                    q_t[r0:r0 + 64, :], start=True, stop=True)
                                nc.scalar.activation(
                                    pt2[:, h_loc * 256:(h_loc + 1) * 256], pss[:],
                                    ACTF.Exp, bias=mb[jt][:], scale=0.125)
                            pT_mp.append(pt2)

                        # replicated row-sums + reciprocal
                        psr = ps.tile([128, 512], F32, name=f"psr_{X}_{b}_{mp}",
                                      tag="ps")
                        nc.tensor.matmul(psr[:], ones128[:], pT_mp[0][:],
                                         start=True, stop=False)
                        nc.tensor.matmul(psr[:], ones128[:], pT_mp[1][:],
                                         start=False, stop=True)
                        rinv = apool.tile([128, 512], F32, name=f"rinv_{X}_{b}_{mp}",
                                          tag="rinv", bufs=2)
                        nc.vector.reciprocal(rinv[:], psr[:])

                        # update^T = v^T @ p (one [64,256] psum per head;
                        # partition-shifted DVE eviction into the pair tile)
                        u_tmp = apool.tile([128, NTOK], F32, name=f"ut_{X}_{b}_{mp}",
                                           tag="utmp", bufs=2)
                        for h_loc in range(2):
                            h = 2 * mp + h_loc
                            psu = ps.tile([64, NTOK], F32,
                                          name=f"psu_{X}_{b}_{mp}_{h_loc}", tag="ps")
                            for jt in range(2):
                                nc.tensor.matmul(
                                    psu[:],
                                    vtok[jt][:, h * 64:(h + 1) * 64],
                                    pT_mp[jt][:, h_loc * 256:(h_loc + 1) * 256],
                                    start=(jt == 0), stop=(jt == 1))
                            r0 = h_loc * 64
                            nc.vector.tensor_tensor(
                                u_tmp[r0:r0 + 64, :], psu[0:64, :],
                                rinv[0:64, h_loc * 256:(h_loc + 1) * 256],
                                ALU.mult)
                        # residual: x^T += u^T (in place, on the idle Pool engine)
                        nc.gpsimd.tensor_tensor(xTraw[:, mp, :], xTraw[:, mp, :],
                                                u_tmp[:], ALU.add)

                    # output projection (w_out resident) -> ACT evict -> DMA out
                    pso = [ps.tile([128, 512], F32, name=f"pso_{X}_{b}_{i}", tag="ps")
                           for i in range(4)]
                    for kt in range(8):
                        wo = apool.tile([128, D], F32R, name=f"wo_{X}_{b}_{kt}",
                                        tag="wo", bufs=3)
                        nc.sync.dma_start(
                            wo[:], wout_d[kt * 128:(kt + 1) * 128, :].bitcast(F32R))
                        for i in range(4):
                            it, ch = divmod(i, 2)
                            nc.tensor.matmul(pso[i][:],
                                             xTraw[:, kt, it * 128:(it + 1) * 128],
                                             wo[:, ch * 512:(ch + 1) * 512],
                                             start=(kt == 0), stop=False)
                    for i in range(4):
                        it, ch = divmod(i, 2)
                        nc.tensor.matmul(pso[i][:], ones1[:],
                                         b_o[:, ch * 512:(ch + 1) * 512],
                                         start=False, stop=True)
                        osb = apool.tile([128, 512], F32, name=f"osb_{X}_{b}_{i}",
                                         tag="osb", bufs=2)
                        nc.scalar.copy(osb[:], pso[i][:])
                        nc.sync.dma_start(
                            out_d[b, it * 128:(it + 1) * 128, ch * 512:(ch + 1) * 512],
                            osb[:])

                if fold_means:
                    # finish v means and compute the v4q gate for branch q
                    recn = apool.tile([4, 1], F32, name="recn_v", tag="recnv", bufs=1)
                    nc.vector.reciprocal(recn[:], accn[:, 0:1])
                    rmean = apool.tile([4, D], F32R, name="rmean_v", tag="rmv",
                                       bufs=1)
                    nc.vector.tensor_scalar(rmean[:], accv[:], recn[:], 0.0,
                                            ALU.mult, ALU.max)
                    mt = wpool.tile([128, 8, 4], F32R, name="meanT_v")
                    for c in range(8):
                        pst = ps.tile([128, 4], F32R, name=f"psmt_v{c}", tag="ps")
                        nc.tensor.transpose(pst[:], rmean[:, c * 128:(c + 1) * 128],
                                            ident[0:4, 0:4])
                        nc.vector.tensor_copy(mt[:, c, :], pst[:])
                    meanT["v"] = mt
                    emit_gate(apool, "v4q", "q", meanT["v"], "a")
            apool_ctx.__exit__(None, None, None)
    nc.compile()
    return nc


_NC = None


def _get_nc():
    global _NC
    if _NC is None:
        _NC = build_nc()
    return _NC


def run(inputs, trace=False):
    nc = _get_nc()
    in_maps = []
    for c in range(NCORES):
        sl = slice(B_CORE * c, B_CORE * (c + 1))
        m = {"v": np.ascontiguousarray(np.asarray(inputs["v"], dtype=np.float32)[sl]),
             "q": np.ascontiguousarray(np.asarray(inputs["q"], dtype=np.float32)[sl]),
             "v_mask": np.ascontiguousarray(
                 np.asarray(inputs["v_mask"], dtype=np.float32)[sl]),
             "q_mask": np.ascontiguousarray(
                 np.asarray(inputs["q_mask"], dtype=np.float32)[sl])}
        for name in WEIGHT_NAMES:
            m[name] = np.ascontiguousarray(np.asarray(inputs[name], dtype=np.float32))
        in_maps.append(m)
    res = run_bass_kernel_spmd(nc, in_maps, core_ids=list(range(NCORES)),
                               trace=trace)
    uv = np.concatenate([res.results[c]["out_v"] for c in range(NCORES)], axis=0)
    uq = np.concatenate([res.results[c]["out_q"] for c in range(NCORES)], axis=0)
    return (uv, uq), res


def kernel(**inputs):
    (uv, uq), _ = run(inputs, trace=False)
    return uv, uq


# revision 25
# speedup vs baseline: 147.9099x; 1.0092x over previous
"""Trainium2 Bass kernel for nn_DyIntraModalityUpdate (dense transformer block).

Strategy: pure data-parallel over batch (B=32 -> 4 per core x 8 cores); each
core computes both the v- and q- branches for its batches. No collectives.

Per-core program:
  prologue: q masked-means -> q4v gate (scales the v branch); prefetch v weights
  branch v main loop (per batch): x^T via PE transposes (raw f32r + relu
    copies); v-projection token-major (bias via K=1 ones-row matmul); k,q
    projections feature-major ((1+gate)^2 folded into k, token-mask into q);
    per-head-pair: scores^T with K=64 row-split, exp on ACT with -1.25e8
    key-mask bias (softmax without max-subtraction, fp32-safe here), replicated
    row-sums via all-ones lhsT matmul, DVE reciprocal, update^T = v^T @ p with
    partition-shifted DVE eviction, residual add on Pool; out-proj token-major.
    v-means accumulate from the same x tiles; the v4q gate computes at branch
    tail so branch q never waits on a separate mean pass.
  branch q main loop: same, using the v4q gate.

All matmuls run in float32r (TF32-like, 1 cycle/row at N>=256, ~1.5e-4 rel
err). fp32r constraints honored: producers write f32r, moving free >= 2, no
PSUM dst base-partition offsets, one accumulation group per PSUM tile.
"""
import os
import sys

import numpy as np

for _p in ("/opt/trn_rl_repo", "/root/.axon_site/_ro/trn_rl_repo"):
    if os.path.isdir(_p) and _p not in sys.path:
        sys.path.insert(0, _p)

import concourse.bass as bass  # noqa: E402,F401
import concourse.mybir as mybir  # noqa: E402
import concourse.tile as tile  # noqa: E402
from concourse import bacc  # noqa: E402
from concourse.bass_utils import run_bass_kernel_spmd  # noqa: E402
from concourse.masks import make_identity  # noqa: E402

F32 = mybir.dt.float32
F32R = mybir.dt.float32r
ALU = mybir.AluOpType
ACTF = mybir.ActivationFunctionType

B_CORE = 4
NTOK = 256
D = 1024
DQKV = 3 * D
NCORES = 8
NEGBIAS = -1e9 / 8.0  # masked_fill(-1e9) then /sqrt(64)

WEIGHT_NAMES = ("w_v4q", "b_v4q", "w_q4v", "b_q4v",
                "w_vlin", "b_vlin", "w_qlin", "b_qlin",
                "w_vout", "b_vout", "w_qout", "b_qout")


def build_nc():
    nc = bacc.Bacc("TRN2", target_bir_lowering=False, debug=False)
    dram = {}

    def din(name, shape):
        dram[name] = nc.dram_tensor(name, shape, F32, kind="ExternalInput").ap()

    def dout(name, shape):
        dram[name] = nc.dram_tensor(name, shape, F32, kind="ExternalOutput").ap()

    din("v", [B_CORE, NTOK, D])
    din("q", [B_CORE, NTOK, D])
    din("v_mask", [B_CORE, NTOK])
    din("q_mask", [B_CORE, NTOK])
    for g in ("v4q", "q4v"):
        din(f"w_{g}", [D, D])
        din(f"b_{g}", [D])
    for x in ("v", "q"):
        din(f"w_{x}lin", [D, DQKV])
        din(f"b_{x}lin", [DQKV])
        din(f"w_{x}out", [D, D])
        din(f"b_{x}out", [D])
    dout("out_v", [B_CORE, NTOK, D])
    dout("out_q", [B_CORE, NTOK, D])

    with tile.TileContext(nc) as tc:
        with tc.tile_pool(name="cpool", bufs=1) as cpool, \
             tc.tile_pool(name="wpool", bufs=1) as wpool, \
             tc.tile_pool(name="pspool", bufs=8, space="PSUM") as ps:
            # ---- constants ----
            ones_f = cpool.tile([128, 128], F32, name="ones_f")
            nc.gpsimd.memset(ones_f[:], 1.0)
            ones128 = cpool.tile([128, 128], F32R, name="ones128")
            nc.vector.tensor_copy(ones128[:], ones_f[:])
            ones1 = cpool.tile([1, 128], F32R, name="ones1")
            nc.vector.tensor_copy(ones1[:], ones_f[0:1, :])
            ident_f = cpool.tile([128, 128], F32, name="ident_f")
            make_identity(nc, ident_f[:])
            ident = cpool.tile([128, 128], F32R, name="ident")
            nc.vector.tensor_copy(ident[:], ident_f[:])
            zero_f = cpool.tile([128, 4], F32, name="zero_f")
            nc.gpsimd.memset(zero_f[:], 0.0)

            bw = {}

            def load_branch_weights(X):
                wlin_d = dram[f"w_{X}lin"]
                blin_d = dram[f"b_{X}lin"]
                bout_d = dram[f"b_{X}out"]
                wout_d = dram[f"w_{X}out"]
                wl = []
                for kt in range(8):
                    t = wpool.tile([128, DQKV], F32R, name=f"wl_{X}_{kt}",
                                   tag=f"wl{kt}", bufs=1)
                    nc.sync.dma_start(
                        t[:], wlin_d[kt * 128:(kt + 1) * 128, :].bitcast(F32R))
                    wl.append(t)
                b_kq = wpool.tile([128, 16], F32, name=f"bkq_{X}", tag="bkq", bufs=1)
                nc.sync.dma_start(b_kq[:],
                                  blin_d[0:2048].rearrange("(o p) -> p o", p=128))
                b_v = wpool.tile([1, D], F32R, name=f"bv_{X}", tag="bv", bufs=1)
                nc.sync.dma_start(b_v[:], blin_d[2048:3072].bitcast(F32R).unsqueeze(0))
                b_o = wpool.tile([1, D], F32R, name=f"bo_{X}", tag="bo", bufs=1)
                nc.sync.dma_start(b_o[:], bout_d.bitcast(F32R).unsqueeze(0))
                bw[X] = (wl, b_kq, b_v, b_o, wout_d)

            meanT = {}
            g2T = {}

            def emit_gate(pool, gname, dst, src_meanT, tag_prefix):
                """gate = sigmoid(relu(mean) @ w + b); store (1+gate)^2
                transposed as g2T[dst] [128, 8, 4] (fp32, per-partition use)."""
                w_d = dram[f"w_{gname}"]
                b_d = dram[f"b_{gname}"]
                bg = pool.tile([1, D], F32R, name=f"bg_{gname}",
                               tag=f"{tag_prefix}bg", bufs=1)
                nc.sync.dma_start(bg[:], b_d.bitcast(F32R).unsqueeze(0))
                gsb = pool.tile([4, D], F32, name=f"g_{gname}",
                                tag=f"{tag_prefix}gsb", bufs=1)
                psg = [ps.tile([4, 512], F32, name=f"psg_{gname}{h}", tag="ps")
                       for h in range(2)]
                for kt in range(8):
                    for h in range(2):
                        wgt = pool.tile([128, 512], F32R,
                                        name=f"wg_{gname}_{kt}_{h}",
                                        tag=f"{tag_prefix}wg",
                                        bufs=(4 if tag_prefix == "p" else 2))
                        nc.sync.dma_start(
                            wgt[:], w_d[kt * 128:(kt + 1) * 128,
                                        h * 512:(h + 1) * 512].bitcast(F32R))
                        nc.tensor.matmul(psg[h][:], src_meanT[:, kt, :], wgt[:],
                                         start=(kt == 0), stop=False)
                for h in range(2):
                    nc.tensor.matmul(psg[h][:], ones1[0:1, 0:4],
                                     bg[:, h * 512:(h + 1) * 512],
                                     start=False, stop=True)
                    nc.scalar.activation(gsb[:, h * 512:(h + 1) * 512], psg[h][:],
                                         ACTF.Sigmoid)
                nc.vector.tensor_scalar_add(gsb[:], gsb[:], 1.0)
                g2 = pool.tile([4, D], F32R, name=f"g2_{gname}",
                               tag=("rmv" if tag_prefix == "a" else f"{tag_prefix}g2"), bufs=1)
                nc.vector.tensor_tensor(g2[:], gsb[:], gsb[:], ALU.mult)
                gt = wpool.tile([128, 8, 4], F32, name=f"g2T_{dst}")
                for c in range(8):
                    pst = ps.tile([128, 4], F32R, name=f"psgt_{gname}{c}", tag="ps")
                    nc.tensor.transpose(pst[:], g2[:, c * 128:(c + 1) * 128],
                                        ident[0:4, 0:4])
                    nc.vector.tensor_copy(gt[:, c, :], pst[:])
                g2T[dst] = gt

            # ---- prologue: q masked-means -> q4v gate (needed by branch v) ----
            with tc.tile_pool(name="propool", bufs=1) as pp:
                m_d = dram["q_mask"]
                x_d = dram["q"]
                ps_mean = [ps.tile([4, 512], F32, name=f"psmean_q{h}", tag="ps")
                           for h in range(2)]
                ps_n = ps.tile([4, 2], F32, name="psn_q", tag="ps")
                for b in range(B_CORE):
                    for jt in range(2):
                        xt = pp.tile([128, D], F32R, name=f"mx_q_{b}_{jt}",
                                     tag="mx", bufs=4)
                        nc.sync.dma_start(
                            xt[:], x_d[b, jt * 128:(jt + 1) * 128, :].bitcast(F32R))
                        mc = pp.tile([128, 4], F32R, name=f"mc_q_{b}_{jt}",
                                     tag="mc", bufs=4)
                        nc.vector.tensor_copy(mc[:], zero_f[:])
                        nc.sync.dma_start(
                            mc[:, b:b + 1],
                            m_d[b, jt * 128:(jt + 1) * 128].bitcast(F32R).unsqueeze(1))
                        first = (b == 0 and jt == 0)
                        last = (b == B_CORE - 1 and jt == 1)
                        for h in range(2):
                            nc.tensor.matmul(ps_mean[h][:], mc[:],
                                             xt[:, h * 512:(h + 1) * 512],
                                             start=first, stop=last)
                        nc.tensor.matmul(ps_n[:], mc[:], ones128[:, 0:2],
                                         start=first, stop=last)
                recn = pp.tile([4, 1], F32, name="recn_q", tag="recn", bufs=1)
                nc.vector.reciprocal(recn[:], ps_n[:, 0:1])
                rmean = pp.tile([4, D], F32R, name="rmean_q", tag="rmean", bufs=1)
                for h in range(2):
                    # relu(masked_sum / n): (psum * recn) max 0
                    nc.vector.tensor_scalar(rmean[:, h * 512:(h + 1) * 512],
                                            ps_mean[h][:], recn[:], 0.0,
                                            ALU.mult, ALU.max)
                mt = wpool.tile([128, 8, 4], F32R, name="meanT_q")
                for c in range(8):
                    pst = ps.tile([128, 4], F32R, name=f"psmt_q{c}", tag="ps")
                    nc.tensor.transpose(pst[:], rmean[:, c * 128:(c + 1) * 128],
                                        ident[0:4, 0:4])
                    nc.vector.tensor_copy(mt[:, c, :], pst[:])
                meanT["q"] = mt

                # q4v gate scales branch v
                emit_gate(pp, "q4v", "v", meanT["q"], "p")
                # prefetch branch-v main weights during gate compute
                load_branch_weights("v")

            # ---- main: per branch ----
            apool_ctx = tc.tile_pool(name="apool", bufs=1)
            apool = apool_ctx.__enter__()
            for X in ("v", "q"):
                gate = g2T[X]
                x_d = dram[X]
                m_d = dram[f"{X}_mask"]
                out_d = dram[f"out_{X}"]
                if X not in bw:
                    load_branch_weights(X)
                wl, b_kq, b_v, b_o, wout_d = bw[X]

                fold_means = (X == "v")
                if fold_means:
                    accv = apool.tile([4, D], F32, name="accv", tag="accv", bufs=1)
                    accn = apool.tile([4, 2], F32, name="accn", tag="accn", bufs=1)

                for b in range(B_CORE):
                    # loads
                    xt = []
                    for jt in range(2):
                        t = apool.tile([128, D], F32R, name=f"x_{X}_{b}_{jt}",
                                       tag="xt", bufs=2)
                        nc.sync.dma_start(
                            t[:], x_d[b, jt * 128:(jt + 1) * 128, :].bitcast(F32R))
                        xt.append(t)
                    mrow = apool.tile([1, NTOK], F32R, name=f"mrow_{X}_{b}",
                                      tag="mrow", bufs=1)
                    nc.sync.dma_start(mrow[:], m_d[b].bitcast(F32R).unsqueeze(0))
                    psmr = ps.tile([128, NTOK], F32, name=f"psmr_{X}_{b}", tag="ps")
                    nc.tensor.matmul(psmr[:], ones1[:], mrow[:], start=True, stop=True)
                    maskrep = apool.tile([128, NTOK], F32, name=f"maskrep_{X}_{b}",
                                         tag="maskrep", bufs=1)
                    nc.vector.tensor_copy(maskrep[:], psmr[:])
                    mb = []
                    for jt in range(2):
                        mcol = apool.tile([128, 1], F32, name=f"mcol_{X}_{b}_{jt}",
                                          tag="mcol", bufs=4)
                        nc.sync.dma_start(
                            mcol[:], m_d[b, jt * 128:(jt + 1) * 128].unsqueeze(1))
                        t = apool.tile([128, 1], F32, name=f"mbias_{X}_{b}_{jt}",
                                       tag="mbias", bufs=4)
                        nc.vector.tensor_scalar(t[:], mcol[:], 1.0, -NEGBIAS,
                                                ALU.subtract, ALU.mult)
                        mb.append(t)

                    if fold_means:
                        # accumulate v masked-sums from this batch's x tiles
                        pm = [ps.tile([4, 512], F32, name=f"pmv_{b}{h}", tag="ps")
                              for h in range(2)]
                        pn = ps.tile([4, 2], F32, name=f"pnv_{b}", tag="ps")
                        for jt in range(2):
                            mc = apool.tile([128, 4], F32R, name=f"mcv_{b}_{jt}",
                                            tag="mcv", bufs=4)
                            nc.vector.tensor_copy(mc[:], zero_f[:])
                            nc.sync.dma_start(
                                mc[:, b:b + 1],
                                m_d[b, jt * 128:(jt + 1) * 128]
                                .bitcast(F32R).unsqueeze(1))
                            for h in range(2):
                                nc.tensor.matmul(pm[h][:], mc[:],
                                                 xt[jt][:, h * 512:(h + 1) * 512],
                                                 start=(jt == 0), stop=(jt == 1))
                            nc.tensor.matmul(pn[:], mc[:], ones128[:, 0:2],
                                             start=(jt == 0), stop=(jt == 1))
                        if b == 0:
                            for h in range(2):
                                nc.vector.tensor_copy(
                                    accv[:, h * 512:(h + 1) * 512], pm[h][:])
                            nc.vector.tensor_copy(accn[:], pn[:])
                        else:
                            for h in range(2):
                                nc.vector.tensor_tensor(
                                    accv[:, h * 512:(h + 1) * 512],
                                    accv[:, h * 512:(h + 1) * 512], pm[h][:],
                                    ALU.add)
                            nc.vector.tensor_tensor(accn[:], accn[:], pn[:], ALU.add)

                    # transpose x -> feature-major raw + relu copies
                    xTraw = apool.tile([128, 8, NTOK], F32R, name=f"xTraw_{X}_{b}",
                                       tag="xTraw", bufs=1)
                    xTrelu = apool.tile([128, 8, NTOK], F32R, name=f"xTrelu_{X}_{b}",
                                        tag="xTrelu", bufs=1)
                    for jt in range(2):
                        for c in range(8):
                            pst = ps.tile([128, 128], F32R,
                                          name=f"pstp_{X}_{b}_{jt}_{c}", tag="ps")
                            nc.tensor.transpose(pst[:],
                                                xt[jt][:, c * 128:(c + 1) * 128],
                                                ident[:])
                            nc.vector.tensor_copy(
                                xTraw[:, c, jt * 128:(jt + 1) * 128], pst[:])
                            nc.scalar.activation(
                                xTrelu[:, c, jt * 128:(jt + 1) * 128], pst[:],
                                ACTF.Relu)

                    # v projection: token-major [tok, dout], bias via ones-row mm
                    vtok = []
                    for jt in range(2):
                        vt = apool.tile([128, D], F32R, name=f"vtok_{X}_{b}_{jt}",
                                        tag=f"vtok{jt}", bufs=1)
                        vtok.append(vt)
                    for jt in range(2):
                        for ch in range(2):
                            psv = ps.tile([128, 512], F32,
                                          name=f"psv_{X}_{b}_{jt}_{ch}", tag="ps")
                            for kt in range(8):
                                nc.tensor.matmul(
                                    psv[:], xTrelu[:, kt, jt * 128:(jt + 1) * 128],
                                    wl[kt][:, 2048 + ch * 512:2048 + (ch + 1) * 512],
                                    start=(kt == 0), stop=False)
                            nc.tensor.matmul(psv[:], ones1[:],
                                             b_v[:, ch * 512:(ch + 1) * 512],
                                             start=False, stop=True)
                            nc.scalar.copy(vtok[jt][:, ch * 512:(ch + 1) * 512],
                                           psv[:])

                    # k,q projections for all head pairs (dense PE phase)
                    k_ts, q_ts = {}, {}
                    for mp in range(8):
                        for part in (mp, 8 + mp):  # k chunk then q chunk
                            psq = ps.tile([128, NTOK], F32,
                                          name=f"pskq_{X}_{b}_{part}", tag="ps")
                            for kt in range(8):
                                nc.tensor.matmul(
                                    psq[:], wl[kt][:, part * 128:(part + 1) * 128],
                                    xTrelu[:, kt, :], start=(kt == 0), stop=(kt == 7))
                            if part < 8:
                                t = apool.tile([128, NTOK], F32R,
                                               name=f"k_{X}_{b}_{mp}", tag=f"k{mp}",
                                               bufs=1)
                                # (psum + bias) * (1+gate)^2  [both per-partition]
                                nc.vector.tensor_scalar(
                                    t[:], psq[:], b_kq[:, part:part + 1],
                                    gate[:, part, b:b + 1], ALU.add, ALU.mult)
                                k_ts[mp] = t
                            else:
                                t = apool.tile([128, NTOK], F32R,
                                               name=f"q_{X}_{b}_{mp}", tag=f"q{mp}",
                                               bufs=1)
                                # (psum + bias) * token_mask  [mask replicated]
                                nc.vector.scalar_tensor_tensor(
                                    t[:], psq[:], b_kq[:, part:part + 1], maskrep[:],
                                    ALU.add, ALU.mult)
                                q_ts[mp] = t

                    # attention per head pair
                    for mp in range(8):
                        k_t, q_t = k_ts[mp], q_ts[mp]
                        pT_mp = []
                        for jt in range(2):
                            pt2 = apool.tile([128, 512], F32R,
                                             name=f"pT_{X}_{b}_{mp}_{jt}", tag="pT",
                                             bufs=4)
                            for h_loc in range(2):
                                r0 = h_loc * 64
                                pss = ps.tile([128, NTOK], F32,
                                              name=f"pss_{X}_{b}_{mp}_{jt}_{h_loc}",
                                              tag="ps")
                                nc.tensor.matmul(
                                    pss[:], k_t[r0:r0 + 64, jt * 128:(jt + 1) * 128],
                                    q_t[r0:r0 + 64, :], start=True, stop=True)
                                nc.scalar.activation(
                                    pt2[:, h_loc * 256:(h_loc + 1) * 256], pss[:],
                                    ACTF.Exp, bias=mb[jt][:], scale=0.125)
                            pT_mp.append(pt2)

                        # replicated row-sums + reciprocal
                        psr = ps.tile([128, 512], F32, name=f"psr_{X}_{b}_{mp}",
                                      tag="ps")
                        nc.tensor.matmul(psr[:], ones128[:], pT_mp[0][:],
                                         start=True, stop=False)
                        nc.tensor.matmul(psr[:], ones128[:], pT_mp[1][:],
                                         start=False, stop=True)
                        rinv = apool.tile([128, 512], F32, name=f"rinv_{X}_{b}_{mp}",
                                          tag="rinv", bufs=2)
                        nc.vector.reciprocal(rinv[:], psr[:])

                        # update^T = v^T @ p (one [64,256] psum per head;
                        # partition-shifted DVE eviction into the pair tile)
                        u_tmp = apool.tile([128, NTOK], F32, name=f"ut_{X}_{b}_{mp}",
                                           tag="utmp", bufs=2)
                        for h_loc in range(2):
                            h = 2 * mp + h_loc
                            psu = ps.tile([64, NTOK], F32,
                                          name=f"psu_{X}_{b}_{mp}_{h_loc}", tag="ps")
                            for jt in range(2):
                                nc.tensor.matmul(
                                    psu[:],
                                    vtok[jt][:, h * 64:(h + 1) * 64],
                                    pT_mp[jt][:, h_loc * 256:(h_loc + 1) * 256],
                                    start=(jt == 0), stop=(jt == 1))
                            r0 = h_loc * 64
                            nc.vector.tensor_tensor(
                                u_tmp[r0:r0 + 64, :], psu[0:64, :],
                                rinv[0:64, h_loc * 256:(h_loc + 1) * 256],
                                ALU.mult)
                        # residual: x^T += u^T (in place, on the idle Pool engine)
                        nc.gpsimd.tensor_tensor(xTraw[:, mp, :], xTraw[:, mp, :],
                                                u_tmp[:], ALU.add)

                    # output projection (w_out resident) -> ACT evict -> DMA out
                    pso = [ps.tile([128, 512], F32, name=f"pso_{X}_{b}_{i}", tag="ps")
                           for i in range(4)]
                    for kt in range(8):
                        wo = apool.tile([128, D], F32R, name=f"wo_{X}_{b}_{kt}",
                                        tag="wo", bufs=3)
                        nc.sync.dma_start(
                            wo[:], wout_d[kt * 128:(kt + 1) * 128, :].bitcast(F32R))
                        for i in range(4):
                            it, ch = divmod(i, 2)
                            nc.tensor.matmul(pso[i][:],
                                             xTraw[:, kt, it * 128:(it + 1) * 128],
                                             wo[:, ch * 512:(ch + 1) * 512],
                                             start=(kt == 0), stop=False)
                    for i in range(4):
                        it, ch = divmod(i, 2)
                        nc.tensor.matmul(pso[i][:], ones1[:],
                                         b_o[:, ch * 512:(ch + 1) * 512],
                                         start=False, stop=True)
                        osb = apool.tile([128, 512], F32, name=f"osb_{X}_{b}_{i}",
                                         tag="osb", bufs=2)
                        nc.scalar.copy(osb[:], pso[i][:])
                        nc.sync.dma_start(
                            out_d[b, it * 128:(it + 1) * 128, ch * 512:(ch + 1) * 512],
                            osb[:])

                if fold_means:
                    # finish v means and compute the v4q gate for branch q
                    recn = apool.tile([4, 1], F32, name="recn_v", tag="recnv", bufs=1)
                    nc.vector.reciprocal(recn[:], accn[:, 0:1])
                    rmean = apool.tile([4, D], F32R, name="rmean_v", tag="rmv",
                                       bufs=1)
                    nc.vector.tensor_scalar(rmean[:], accv[:], recn[:], 0.0,
                                            ALU.mult, ALU.max)
                    mt = wpool.tile([128, 8, 4], F32R, name="meanT_v")
                    for c in range(8):
                        pst = ps.tile([128, 4], F32R, name=f"psmt_v{c}", tag="ps")
                        nc.tensor.transpose(pst[:], rmean[:, c * 128:(c + 1) * 128],
                                            ident[0:4, 0:4])
                        nc.vector.tensor_copy(mt[:, c, :], pst[:])
                    meanT["v"] = mt
                    emit_gate(apool, "v4q", "q", meanT["v"], "a")
            apool_ctx.__exit__(None, None, None)
    nc.compile()
    return nc


_NC = None


def _get_nc():
    global _NC
    if _NC is None:
        _NC = build_nc()
    return _NC


def run(inputs, trace=False):
    nc = _get_nc()
    in_maps = []
    for c in range(NCORES):
        sl = slice(B_CORE * c, B_CORE * (c + 1))
        m = {"v": np.ascontiguousarray(np.asarray(inputs["v"], dtype=np.float32)[sl]),
             "q": np.ascontiguousarray(np.asarray(inputs["q"], dtype=np.float32)[sl]),
             "v_mask": np.ascontiguousarray(
                 np.asarray(inputs["v_mask"], dtype=np.float32)[sl]),
             "q_mask": np.ascontiguousarray(
                 np.asarray(inputs["q_mask"], dtype=np.float32)[sl])}
        for name in WEIGHT_NAMES:
            m[name] = np.ascontiguousarray(np.asarray(inputs[name], dtype=np.float32))
        in_maps.append(m)
    res = run_bass_kernel_spmd(nc, in_maps, core_ids=list(range(NCORES)),
                               trace=trace)
    uv = np.concatenate([res.results[c]["out_v"] for c in range(NCORES)], axis=0)
    uq = np.concatenate([res.results[c]["out_q"] for c in range(NCORES)], axis=0)
    return (uv, uq), res


def kernel(**inputs):
    (uv, uq), _ = run(inputs, trace=False)
    return uv, uq


# revision 29
# speedup vs baseline: 147.9198x; 1.0001x over previous
"""Trainium2 Bass kernel for nn_DyIntraModalityUpdate (dense transformer block).

Strategy: pure data-parallel over batch (B=32 -> 4 per core x 8 cores); each
core computes both the v- and q- branches for its batches. No collectives.

Per-core program:
  prologue: q masked-means -> q4v gate (scales the v branch); prefetch v weights
  branch v main loop (per batch): x^T via PE transposes (raw f32r + relu
    copies); v-projection token-major (bias via K=1 ones-row matmul); k,q
    projections feature-major ((1+gate)^2 folded into k, token-mask into q);
    per-head-pair: scores^T with K=64 row-split, exp on ACT with -1.25e8
    key-mask bias (softmax without max-subtraction, fp32-safe here), replicated
    row-sums via all-ones lhsT matmul, DVE reciprocal, update^T = v^T @ p with
    partition-shifted DVE eviction, residual add on Pool; out-proj token-major.
    v-means accumulate from the same x tiles; the v4q gate computes at branch
    tail so branch q never waits on a separate mean pass.
  branch q main loop: same, using the v4q gate.

All matmuls run in float32r (TF32-like, 1 cycle/row at N>=256, ~1.5e-4 rel
err). fp32r constraints honored: producers write f32r, moving free >= 2, no
PSUM dst base-partition offsets, one accumulation group per PSUM tile.
"""
import os
import sys

import numpy as np

for _p in ("/opt/trn_rl_repo", "/root/.axon_site/_ro/trn_rl_repo"):
    if os.path.isdir(_p) and _p not in sys.path:
        sys.path.insert(0, _p)

import concourse.bass as bass  # noqa: E402,F401
import concourse.mybir as mybir  # noqa: E402
import concourse.tile as tile  # noqa: E402
from concourse import bacc  # noqa: E402
from concourse.bass_utils import run_bass_kernel_spmd  # noqa: E402
from concourse.masks import make_identity  # noqa: E402

F32 = mybir.dt.float32
F32R = mybir.dt.float32r
ALU = mybir.AluOpType
ACTF = mybir.ActivationFunctionType

B_CORE = 4
NTOK = 256
D = 1024
DQKV = 3 * D
NCORES = 8
NEGBIAS = -1e9 / 8.0  # masked_fill(-1e9) then /sqrt(64)

WEIGHT_NAMES = ("w_v4q", "b_v4q", "w_q4v", "b_q4v",
                "w_vlin", "b_vlin", "w_qlin", "b_qlin",
                "w_vout", "b_vout", "w_qout", "b_qout")


def build_nc():
    nc = bacc.Bacc("TRN2", target_bir_lowering=False, debug=False)
    dram = {}

    def din(name, shape):
        dram[name] = nc.dram_tensor(name, shape, F32, kind="ExternalInput").ap()

    def dout(name, shape):
        dram[name] = nc.dram_tensor(name, shape, F32, kind="ExternalOutput").ap()

    din("v", [B_CORE, NTOK, D])
    din("q", [B_CORE, NTOK, D])
    din("v_mask", [B_CORE, NTOK])
    din("q_mask", [B_CORE, NTOK])
    for g in ("v4q", "q4v"):
        din(f"w_{g}", [D, D])
        din(f"b_{g}", [D])
    for x in ("v", "q"):
        din(f"w_{x}lin", [D, DQKV])
        din(f"b_{x}lin", [DQKV])
        din(f"w_{x}out", [D, D])
        din(f"b_{x}out", [D])
    dout("out_v", [B_CORE, NTOK, D])
    dout("out_q", [B_CORE, NTOK, D])

    with tile.TileContext(nc) as tc:
        with tc.tile_pool(name="cpool", bufs=1) as cpool, \
             tc.tile_pool(name="wpool", bufs=1) as wpool, \
             tc.tile_pool(name="pspool", bufs=8, space="PSUM") as ps:
            # ---- constants ----
            ones_f = cpool.tile([128, 128], F32, name="ones_f")
            nc.gpsimd.memset(ones_f[:], 1.0)
            ones128 = cpool.tile([128, 128], F32R, name="ones128")
            nc.vector.tensor_copy(ones128[:], ones_f[:])
            ones1 = cpool.tile([1, 128], F32R, name="ones1")
            nc.vector.tensor_copy(ones1[:], ones_f[0:1, :])
            ident_f = cpool.tile([128, 128], F32, name="ident_f")
            make_identity(nc, ident_f[:])
            ident = cpool.tile([128, 128], F32R, name="ident")
            nc.vector.tensor_copy(ident[:], ident_f[:])
            zero_f = cpool.tile([128, 4], F32, name="zero_f")
            nc.gpsimd.memset(zero_f[:], 0.0)

            bw = {}

            def load_branch_weights(X):
                wlin_d = dram[f"w_{X}lin"]
                blin_d = dram[f"b_{X}lin"]
                bout_d = dram[f"b_{X}out"]
                wout_d = dram[f"w_{X}out"]
                wl = []
                for kt in range(8):
                    t = wpool.tile([128, DQKV], F32R, name=f"wl_{X}_{kt}",
                                   tag=f"wl{kt}", bufs=1)
                    nc.sync.dma_start(
                        t[:], wlin_d[kt * 128:(kt + 1) * 128, :].bitcast(F32R))
                    wl.append(t)
                b_kq = wpool.tile([128, 16], F32, name=f"bkq_{X}", tag="bkq", bufs=1)
                nc.sync.dma_start(b_kq[:],
                                  blin_d[0:2048].rearrange("(o p) -> p o", p=128))
                b_v = wpool.tile([1, D], F32R, name=f"bv_{X}", tag="bv", bufs=1)
                nc.sync.dma_start(b_v[:], blin_d[2048:3072].bitcast(F32R).unsqueeze(0))
                b_o = wpool.tile([1, D], F32R, name=f"bo_{X}", tag="bo", bufs=1)
                nc.sync.dma_start(b_o[:], bout_d.bitcast(F32R).unsqueeze(0))
                bw[X] = (wl, b_kq, b_v, b_o, wout_d)

            meanT = {}
            g2T = {}

            def emit_gate(pool, gname, dst, src_meanT, tag_prefix):
                """gate = sigmoid(relu(mean) @ w + b); store (1+gate)^2
                transposed as g2T[dst] [128, 8, 4] (fp32, per-partition use)."""
                w_d = dram[f"w_{gname}"]
                b_d = dram[f"b_{gname}"]
                bg = pool.tile([1, D], F32R, name=f"bg_{gname}",
                               tag=f"{tag_prefix}bg", bufs=1)
                nc.sync.dma_start(bg[:], b_d.bitcast(F32R).unsqueeze(0))
                gsb = pool.tile([4, D], F32, name=f"g_{gname}",
                                tag=f"{tag_prefix}gsb", bufs=1)
                psg = [ps.tile([4, 512], F32, name=f"psg_{gname}{h}", tag="ps")
                       for h in range(2)]
                for kt in range(8):
                    for h in range(2):
                        wgt = pool.tile([128, 512], F32R,
                                        name=f"wg_{gname}_{kt}_{h}",
                                        tag=f"{tag_prefix}wg",
                                        bufs=(4 if tag_prefix == "p" else 2))
                        nc.sync.dma_start(
                            wgt[:], w_d[kt * 128:(kt + 1) * 128,
                                        h * 512:(h + 1) * 512].bitcast(F32R))
                        nc.tensor.matmul(psg[h][:], src_meanT[:, kt, :], wgt[:],
                                         start=(kt == 0), stop=False)
                for h in range(2):
                    nc.tensor.matmul(psg[h][:], ones1[0:1, 0:4],
                                     bg[:, h * 512:(h + 1) * 512],
                                     start=False, stop=True)
                    nc.scalar.activation(gsb[:, h * 512:(h + 1) * 512], psg[h][:],
                                         ACTF.Sigmoid)
                nc.vector.tensor_scalar_add(gsb[:], gsb[:], 1.0)
                g2 = pool.tile([4, D], F32R, name=f"g2_{gname}",
                               tag=("rmv" if tag_prefix == "a" else f"{tag_prefix}g2"), bufs=1)
                nc.vector.tensor_tensor(g2[:], gsb[:], gsb[:], ALU.mult)
                gt = wpool.tile([128, 8, 4], F32, name=f"g2T_{dst}")
                for c in range(8):
                    pst = ps.tile([128, 4], F32R, name=f"psgt_{gname}{c}", tag="ps")
                    nc.tensor.transpose(pst[:], g2[:, c * 128:(c + 1) * 128],
                                        ident[0:4, 0:4])
                    nc.vector.tensor_copy(gt[:, c, :], pst[:])
                g2T[dst] = gt

            # ---- prologue: q masked-means -> q4v gate (needed by branch v) ----
            with tc.tile_pool(name="propool", bufs=1) as pp:
                m_d = dram["q_mask"]
                x_d = dram["q"]
                ps_mean = [ps.tile([4, 512], F32, name=f"psmean_q{h}", tag="ps")
                           for h in range(2)]
                ps_n = ps.tile([4, 2], F32, name="psn_q", tag="ps")
                for b in range(B_CORE):
                    for jt in range(2):
                        xt = pp.tile([128, D], F32R, name=f"mx_q_{b}_{jt}",
                                     tag="mx", bufs=4)
                        nc.sync.dma_start(
                            xt[:], x_d[b, jt * 128:(jt + 1) * 128, :].bitcast(F32R))
                        mc = pp.tile([128, 4], F32R, name=f"mc_q_{b}_{jt}",
                                     tag="mc", bufs=4)
                        nc.vector.tensor_copy(mc[:], zero_f[:])
                        nc.sync.dma_start(
                            mc[:, b:b + 1],
                            m_d[b, jt * 128:(jt + 1) * 128].bitcast(F32R).unsqueeze(1))
                        first = (b == 0 and jt == 0)
                        last = (b == B_CORE - 1 and jt == 1)
                        for h in range(2):
                            nc.tensor.matmul(ps_mean[h][:], mc[:],
                                             xt[:, h * 512:(h + 1) * 512],
                                             start=first, stop=last)
                        nc.tensor.matmul(ps_n[:], mc[:], ones128[:, 0:2],
                                         start=first, stop=last)
                recn = pp.tile([4, 1], F32, name="recn_q", tag="recn", bufs=1)
                nc.vector.reciprocal(recn[:], ps_n[:, 0:1])
                rmean = pp.tile([4, D], F32R, name="rmean_q", tag="rmean", bufs=1)
                for h in range(2):
                    # relu(masked_sum / n): (psum * recn) max 0
                    nc.vector.tensor_scalar(rmean[:, h * 512:(h + 1) * 512],
                                            ps_mean[h][:], recn[:], 0.0,
                                            ALU.mult, ALU.max)
                mt = wpool.tile([128, 8, 4], F32R, name="meanT_q")
                for c in range(8):
                    pst = ps.tile([128, 4], F32R, name=f"psmt_q{c}", tag="ps")
                    nc.tensor.transpose(pst[:], rmean[:, c * 128:(c + 1) * 128],
                                        ident[0:4, 0:4])
                    nc.vector.tensor_copy(mt[:, c, :], pst[:])
                meanT["q"] = mt

                # q4v gate scales branch v
                emit_gate(pp, "q4v", "v", meanT["q"], "p")
                # prefetch branch-v main weights during gate compute
                load_branch_weights("v")

            # ---- main: per branch ----
            apool_ctx = tc.tile_pool(name="apool", bufs=1)
            apool = apool_ctx.__enter__()
            for X in ("v", "q"):
                gate = g2T[X]
                x_d = dram[X]
                m_d = dram[f"{X}_mask"]
                out_d = dram[f"out_{X}"]
                if X not in bw:
                    load_branch_weights(X)
                wl, b_kq, b_v, b_o, wout_d = bw[X]

                fold_means = (X == "v")
                if fold_means:
                    accv = apool.tile([4, D], F32, name="accv", tag="accv", bufs=1)
                    accn = apool.tile([4, 2], F32, name="accn", tag="accn", bufs=1)

                for b in range(B_CORE):
                    # loads
                    xt = []
                    for jt in range(2):
                        t = apool.tile([128, D], F32R, name=f"x_{X}_{b}_{jt}",
                                       tag="xt", bufs=2)
                        nc.sync.dma_start(
                            t[:], x_d[b, jt * 128:(jt + 1) * 128, :].bitcast(F32R))
                        xt.append(t)
                    mrow = apool.tile([1, NTOK], F32R, name=f"mrow_{X}_{b}",
                                      tag="mrow", bufs=1)
                    nc.sync.dma_start(mrow[:], m_d[b].bitcast(F32R).unsqueeze(0))
                    psmr = ps.tile([128, NTOK], F32, name=f"psmr_{X}_{b}", tag="ps")
                    nc.tensor.matmul(psmr[:], ones1[:], mrow[:], start=True, stop=True)
                    maskrep = apool.tile([128, NTOK], F32, name=f"maskrep_{X}_{b}",
                                         tag="maskrep", bufs=1)
                    nc.vector.tensor_copy(maskrep[:], psmr[:])
                    mb = []
                    for jt in range(2):
                        mcol = apool.tile([128, 1], F32, name=f"mcol_{X}_{b}_{jt}",
                                          tag="mcol", bufs=8)
                        nc.sync.dma_start(
                            mcol[:], m_d[b, jt * 128:(jt + 1) * 128].unsqueeze(1))
                        t = apool.tile([128, 1], F32, name=f"mbias_{X}_{b}_{jt}",
                                       tag="mbias", bufs=8)
                        nc.vector.tensor_scalar(t[:], mcol[:], 1.0, -NEGBIAS,
                                                ALU.subtract, ALU.mult)
                        mb.append(t)

                    if fold_means:
                        # accumulate v masked-sums from this batch's x tiles
                        pm = [ps.tile([4, 512], F32, name=f"pmv_{b}{h}", tag="ps")
                              for h in range(2)]
                        pn = ps.tile([4, 2], F32, name=f"pnv_{b}", tag="ps")
                        for jt in range(2):
                            mc = apool.tile([128, 4], F32R, name=f"mcv_{b}_{jt}",
                                            tag="mcv", bufs=4)
                            nc.vector.tensor_copy(mc[:], zero_f[:])
                            nc.sync.dma_start(
                                mc[:, b:b + 1],
                                m_d[b, jt * 128:(jt + 1) * 128]
                                .bitcast(F32R).unsqueeze(1))
                            for h in range(2):
                                nc.tensor.matmul(pm[h][:], mc[:],
                                                 xt[jt][:, h * 512:(h + 1) * 512],
                                                 start=(jt == 0), stop=(jt == 1))
                            nc.tensor.matmul(pn[:], mc[:], ones128[:, 0:2],
                                             start=(jt == 0), stop=(jt == 1))
                        if b == 0:
                            for h in range(2):
                                nc.vector.tensor_copy(
                                    accv[:, h * 512:(h + 1) * 512], pm[h][:])
                            nc.vector.tensor_copy(accn[:], pn[:])
                        else:
                            for h in range(2):
                                nc.vector.tensor_tensor(
                                    accv[:, h * 512:(h + 1) * 512],
                                    accv[:, h * 512:(h + 1) * 512], pm[h][:],
                                    ALU.add)
                            nc.vector.tensor_tensor(accn[:], accn[:], pn[:], ALU.add)

                    # transpose x -> feature-major raw + relu copies
                    xTraw = apool.tile([128, 8, NTOK], F32R, name=f"xTraw_{X}_{b}",
                                       tag="xTraw", bufs=1)
                    xTrelu = apool.tile([128, 8, NTOK], F32R, name=f"xTrelu_{X}_{b}",
                                        tag="xTrelu", bufs=1)
                    for jt in range(2):
                        for c in range(8):
                            pst = ps.tile([128, 128], F32R,
                                          name=f"pstp_{X}_{b}_{jt}_{c}", tag="ps")
                            nc.tensor.transpose(pst[:],
                                                xt[jt][:, c * 128:(c + 1) * 128],
                                                ident[:])
                            nc.vector.tensor_copy(
                                xTraw[:, c, jt * 128:(jt + 1) * 128], pst[:])
                            nc.scalar.activation(
                                xTrelu[:, c, jt * 128:(jt + 1) * 128], pst[:],
                                ACTF.Relu)

                    # v projection: token-major [tok, dout], bias via ones-row mm
                    vtok = []
                    for jt in range(2):
                        vt = apool.tile([128, D], F32R, name=f"vtok_{X}_{b}_{jt}",
                                        tag=f"vtok{jt}", bufs=1)
                        vtok.append(vt)
                    for jt in range(2):
                        for ch in range(2):
                            psv = ps.tile([128, 512], F32,
                                          name=f"psv_{X}_{b}_{jt}_{ch}", tag="ps")
                            for kt in range(8):
                                nc.tensor.matmul(
                                    psv[:], xTrelu[:, kt, jt * 128:(jt + 1) * 128],
                                    wl[kt][:, 2048 + ch * 512:2048 + (ch + 1) * 512],
                                    start=(kt == 0), stop=False)
                            nc.tensor.matmul(psv[:], ones1[:],
                                             b_v[:, ch * 512:(ch + 1) * 512],
                                             start=False, stop=True)
                            nc.scalar.copy(vtok[jt][:, ch * 512:(ch + 1) * 512],
                                           psv[:])

                    # k,q projections for all head pairs (dense PE phase)
                    k_ts, q_ts = {}, {}
                    for mp in range(8):
                        for part in (mp, 8 + mp):  # k chunk then q chunk
                            psq = ps.tile([128, NTOK], F32,
                                          name=f"pskq_{X}_{b}_{part}", tag="ps")
                            for kt in range(8):
                                nc.tensor.matmul(
                                    psq[:], wl[kt][:, part * 128:(part + 1) * 128],
                                    xTrelu[:, kt, :], start=(kt == 0), stop=(kt == 7))
                            if part < 8:
                                t = apool.tile([128, NTOK], F32R,
                                               name=f"k_{X}_{b}_{mp}", tag=f"k{mp}",
                                               bufs=1)
                                # (psum + bias) * (1+gate)^2  [both per-partition]
                                nc.vector.tensor_scalar(
                                    t[:], psq[:], b_kq[:, part:part + 1],
                                    gate[:, part, b:b + 1], ALU.add, ALU.mult)
                                k_ts[mp] = t
                            else:
                                t = apool.tile([128, NTOK], F32R,
                                               name=f"q_{X}_{b}_{mp}", tag=f"q{mp}",
                                               bufs=1)
                                # (psum + bias) * token_mask  [mask replicated]
                                nc.vector.scalar_tensor_tensor(
                                    t[:], psq[:], b_kq[:, part:part + 1], maskrep[:],
                                    ALU.add, ALU.mult)
                                q_ts[mp] = t

                    # attention per head pair
                    for mp in range(8):
                        k_t, q_t = k_ts[mp], q_ts[mp]
                        pT_mp = []
                        for jt in range(2):
                            pt2 = apool.tile([128, 512], F32R,
                                             name=f"pT_{X}_{b}_{mp}_{jt}", tag="pT",
                                             bufs=4)
                            for h_loc in range(2):
                                r0 = h_loc * 64
                                pss = ps.tile([128, NTOK], F32,
                                              name=f"pss_{X}_{b}_{mp}_{jt}_{h_loc}",
                                              tag="ps")
                                nc.tensor.matmul(
                                    pss[:], k_t[r0:r0 + 64, jt * 128:(jt + 1) * 128],
                                    q_t[r0:r0 + 64, :], start=True, stop=True)
                                nc.scalar.activation(
                                    pt2[:, h_loc * 256:(h_loc + 1) * 256], pss[:],
                                    ACTF.Exp, bias=mb[jt][:], scale=0.125)
                            pT_mp.append(pt2)

                        # replicated row-sums + reciprocal
                        psr = ps.tile([128, 512], F32, name=f"psr_{X}_{b}_{mp}",
                                      tag="ps")
                        nc.tensor.matmul(psr[:], ones128[:], pT_mp[0][:],
                                         start=True, stop=False)
                        nc.tensor.matmul(psr[:], ones128[:], pT_mp[1][:],
                                         start=False, stop=True)
                        rinv = apool.tile([128, 512], F32, name=f"rinv_{X}_{b}_{mp}",
                                          tag="rinv", bufs=2)
                        nc.vector.reciprocal(rinv[:], psr[:])

                        # update^T = v^T @ p (one [64,256] psum per head;
                        # partition-shifted DVE eviction into the pair tile)
                        u_tmp = apool.tile([128, NTOK], F32, name=f"ut_{X}_{b}_{mp}",
                                           tag="utmp", bufs=2)
                        for h_loc in range(2):
                            h = 2 * mp + h_loc
                            psu = ps.tile([64, NTOK], F32,
                                          name=f"psu_{X}_{b}_{mp}_{h_loc}", tag="ps")
                            for jt in range(2):
                                nc.tensor.matmul(
                                    psu[:],
                                    vtok[jt][:, h * 64:(h + 1) * 64],
                                    pT_mp[jt][:, h_loc * 256:(h_loc + 1) * 256],
                                    start=(jt == 0), stop=(jt == 1))
                            r0 = h_loc * 64
                            nc.vector.tensor_tensor(
                                u_tmp[r0:r0 + 64, :], psu[0:64, :],
                                rinv[0:64, h_loc * 256:(h_loc + 1) * 256],
                                ALU.mult)
                        # residual: x^T += u^T (in place, on the idle Pool engine)
                        nc.gpsimd.tensor_tensor(xTraw[:, mp, :], xTraw[:, mp, :],
                                                u_tmp[:], ALU.add)

                    # output projection (w_out resident) -> ACT evict -> DMA out
                    pso = [ps.tile([128, 512], F32, name=f"pso_{X}_{b}_{i}", tag="ps")
                           for i in range(4)]
                    for kt in range(8):
                        wo = apool.tile([128, D], F32R, name=f"wo_{X}_{b}_{kt}",
                                        tag="wo", bufs=3)
                        nc.sync.dma_start(
                            wo[:], wout_d[kt * 128:(kt + 1) * 128, :].bitcast(F32R))
                        for i in range(4):
                            it, ch = divmod(i, 2)
                            nc.tensor.matmul(pso[i][:],
                                             xTraw[:, kt, it * 128:(it + 1) * 128],
                                             wo[:, ch * 512:(ch + 1) * 512],
                                             start=(kt == 0), stop=False)
                    for i in range(4):
                        it, ch = divmod(i, 2)
                        nc.tensor.matmul(pso[i][:], ones1[:],
                                         b_o[:, ch * 512:(ch + 1) * 512],
                                         start=False, stop=True)
                        osb = apool.tile([128, 512], F32, name=f"osb_{X}_{b}_{i}",
                                         tag="osb", bufs=2)
                        nc.scalar.copy(osb[:], pso[i][:])
                        nc.sync.dma_start(
                            out_d[b, it * 128:(it + 1) * 128, ch * 512:(ch + 1) * 512],
                            osb[:])

                if fold_means:
                    # finish v means and compute the v4q gate for branch q
                    recn = apool.tile([4, 1], F32, name="recn_v", tag="recnv", bufs=1)
                    nc.vector.reciprocal(recn[:], accn[:, 0:1])
                    rmean = apool.tile([4, D], F32R, name="rmean_v", tag="rmv",
                                       bufs=1)
                    nc.vector.tensor_scalar(rmean[:], accv[:], recn[:], 0.0,
                                            ALU.mult, ALU.max)
                    mt = wpool.tile([128, 8, 4], F32R, name="meanT_v")
                    for c in range(8):
                        pst = ps.tile([128, 4], F32R, name=f"psmt_v{c}", tag="ps")
                        nc.tensor.transpose(pst[:], rmean[:, c * 128:(c + 1) * 128],
                                            ident[0:4, 0:4])
                        nc.vector.tensor_copy(mt[:, c, :], pst[:])
                    meanT["v"] = mt
                    emit_gate(apool, "v4q", "q", meanT["v"], "a")
            apool_ctx.__exit__(None, None, None)
    nc.compile()
    return nc


_NC = None


def _get_nc():
    global _NC
    if _NC is None:
        _NC = build_nc()
    return _NC


def run(inputs, trace=False):
    nc = _get_nc()
    in_maps = []
    for c in range(NCORES):
        sl = slice(B_CORE * c, B_CORE * (c + 1))
        m = {"v": np.ascontiguousarray(np.asarray(inputs["v"], dtype=np.float32)[sl]),
             "q": np.ascontiguousarray(np.asarray(inputs["q"], dtype=np.float32)[sl]),
             "v_mask": np.ascontiguousarray(
                 np.asarray(inputs["v_mask"], dtype=np.float32)[sl]),
             "q_mask": np.ascontiguousarray(
                 np.asarray(inputs["q_mask"], dtype=np.float32)[sl])}
        for name in WEIGHT_NAMES:
            m[name] = np.ascontiguousarray(np.asarray(inputs[name], dtype=np.float32))
        in_maps.append(m)
    res = run_bass_kernel_spmd(nc, in_maps, core_ids=list(range(NCORES)),
                               trace=trace)
    uv = np.concatenate([res.results[c]["out_v"] for c in range(NCORES)], axis=0)
    uq = np.concatenate([res.results[c]["out_q"] for c in range(NCORES)], axis=0)
    return (uv, uq), res


def kernel(**inputs):
    (uv, uq), _ = run(inputs, trace=False)
    return uv, uq
